# revision 2
# baseline (speedup 1.0000x reference)
"""Cross-attention layer on 8 trn2 NeuronCores, data-parallel over batch.

Problem (hardcoded): B=8, S1=S2=2048, D=512, fp32.
  q = x1 @ Wq.T + bq ; k = x2 @ Wk.T + bk ; v = x2 @ Wv.T + bv
  out = softmax(q k^T / D) @ v

Sharding: batch b -> core b. Each core runs the full attention for one
batch element; no collectives. Host-side prep is layout only (transpose
+ bf16 cast); all math runs on device. Matmul operands are bf16 (fp32
PSUM accumulation); softmax statistics and output are fp32.

Layouts per core (partition dim first):
  x1t/x2t  [D, S]  bf16   d-on-partitions (TensorE contracts partitions)
  wqt/wkt/wvt [D, D] bf16 (= W.T, so [d, e])
  QT, KT   [D, S]  bf16   from matmul(lhsT=wqt_chunk, rhs=x1t)
  V        [S2, D] bf16   from matmul(lhsT=x2t_chunk, rhs=wvt)
  scores block [128 s, 2048 t] PSUM fp32; exp on ScalarE with fused
  per-partition accum_out row-sums; attn bf16; attn^T via PE transpose;
  out block [128 s, 512 e] = attn^T-matmul against V, scaled by
  1/rowsum and biased by bv in one DVE scalar_tensor_tensor.
"""

import numpy as np
import ml_dtypes

import concourse.bass as bass
import concourse.mybir as mybir
import concourse.tile as tile
from concourse import bacc
from concourse.bass import ts
from concourse.bass_utils import run_bass_kernel_spmd
from concourse.masks import make_identity

B, S1, S2, D = 8, 2048, 2048, 512
N_CORES = 8
P = 128
DC = D // P      # 4 chunks of the d/e dims
NT = S2 // P     # 16 key/value 128-chunks
NS = S1 // P     # 16 query 128-blocks
NG = S2 // 512   # 4 key 512-groups
SG = S1 // 512   # 4 query 512-groups

FP32 = mybir.dt.float32
BF16 = mybir.dt.bfloat16
AF = mybir.ActivationFunctionType


def build_nc():
    nc = bacc.Bacc(None, target_bir_lowering=False, debug=False, num_devices=N_CORES)

    x1t_d = nc.dram_tensor("x1t", [D, S1], BF16, kind="ExternalInput")
    x2t_d = nc.dram_tensor("x2t", [D, S2], BF16, kind="ExternalInput")
    wqt_d = nc.dram_tensor("wqt", [D, D], BF16, kind="ExternalInput")
    wkt_d = nc.dram_tensor("wkt", [D, D], BF16, kind="ExternalInput")
    wvt_d = nc.dram_tensor("wvt", [D, D], BF16, kind="ExternalInput")
    bqs_d = nc.dram_tensor("bqs", [P, DC], FP32, kind="ExternalInput")
    bks_d = nc.dram_tensor("bks", [P, DC], FP32, kind="ExternalInput")
    bvb_d = nc.dram_tensor("bvb", [P, D], FP32, kind="ExternalInput")
    out_d = nc.dram_tensor("out", [S1, D], FP32, kind="ExternalOutput")

    with tile.TileContext(nc) as tc:
        with (
            tc.tile_pool(name="const", bufs=1) as const,
            tc.tile_pool(name="xin", bufs=1) as xin,
            tc.tile_pool(name="proj", bufs=1) as proj,
            tc.tile_pool(name="apool", bufs=2) as apool,
            tc.tile_pool(name="tpool", bufs=4) as tpool,
            tc.tile_pool(name="opool", bufs=2) as opool,
            tc.tile_pool(name="rpool", bufs=2) as rpool,
            tc.tile_pool(name="psA", bufs=2, space="PSUM") as psA,
            tc.tile_pool(name="psS", bufs=1, space="PSUM") as psS,
            tc.tile_pool(name="psT", bufs=2, space="PSUM") as psT,
        ):
            ident = const.tile([P, P], BF16, tag="ident")
            make_identity(nc, ident)
            bqs = const.tile([P, DC], FP32, tag="bqs")
            nc.sync.dma_start(bqs[:], bqs_d[:])
            bks = const.tile([P, DC], FP32, tag="bks")
            nc.sync.dma_start(bks[:], bks_d[:])
            bvb = const.tile([P, D], FP32, tag="bvb")
            nc.sync.dma_start(bvb[:], bvb_d[:])

            wq, wk, wv, x1t, x2t = [], [], [], [], []
            for c in range(DC):
                for lst, tname, dram in (
                    (wq, "wq", wqt_d), (wk, "wk", wkt_d), (wv, "wv", wvt_d),
                ):
                    t = const.tile([P, D], BF16, tag=f"{tname}{c}")
                    nc.sync.dma_start(t[:], dram[ts(c, P), :])
                    lst.append(t)
                t = xin.tile([P, S1], BF16, tag=f"x1t{c}")
                nc.sync.dma_start(t[:], x1t_d[ts(c, P), :])
                x1t.append(t)
                t = xin.tile([P, S2], BF16, tag=f"x2t{c}")
                nc.sync.dma_start(t[:], x2t_d[ts(c, P), :])
                x2t.append(t)

            qt = [proj.tile([P, S1], BF16, tag=f"qt{e}", name=f"qt{e}") for e in range(DC)]
            kt = [proj.tile([P, S2], BF16, tag=f"kt{e}", name=f"kt{e}") for e in range(DC)]
            v = [proj.tile([P, D], BF16, tag=f"v{t}", name=f"v{t}") for t in range(NT)]

            # QT[e, s] / KT[e, t] projections: lhsT = wt[d, e], rhs = xt[d, s]
            for xt, wt, bt, dst in ((x1t, wq, bqs, qt), (x2t, wk, bks, kt)):
                for e in range(DC):
                    for g in range(SG):
                        ps = psA.tile([P, 512], FP32, tag="psA")
                        for d in range(DC):
                            nc.tensor.matmul(
                                ps[:], wt[d][:, ts(e, P)], xt[d][:, ts(g, 512)],
                                start=(d == 0), stop=(d == DC - 1),
                            )
                        nc.scalar.activation(
                            dst[e][:, ts(g, 512)], ps[:], AF.Identity,
                            bias=bt[:, e:e + 1], scale=1.0,
                        )
            # V[t, e]: lhsT = x2t[d, t-chunk], rhs = wvt[d, e].  bv is
            # folded into the final output (attn rows sum to 1).
            for t in range(NT):
                ps = psA.tile([P, 512], FP32, tag="psA")
                for d in range(DC):
                    nc.tensor.matmul(
                        ps[:], x2t[d][:, ts(t, P)], wv[d][:],
                        start=(d == 0), stop=(d == DC - 1),
                    )
                nc.scalar.copy(v[t][:], ps[:])

            # attention, one 128-row query block at a time
            for i in range(NS):
                ps_s = psS.tile([P, S2], FP32, tag="scores")
                attn = apool.tile([P, S2], BF16, tag="attn")
                rs = rpool.tile([P, NG], FP32, tag="rs")
                for g in range(NG):
                    for e in range(DC):
                        nc.tensor.matmul(
                            ps_s[:, ts(g, 512)],
                            qt[e][:, ts(i, P)], kt[e][:, ts(g, 512)],
                            start=(e == 0), stop=(e == DC - 1),
                        )
                    # scores are O(+-0.25) after the 1/D scale: exp needs
                    # no max-subtraction.  accum_out = fused row sums.
                    nc.scalar.activation(
                        attn[:, ts(g, 512)], ps_s[:, ts(g, 512)], AF.Exp,
                        scale=1.0 / D, accum_out=rs[:, g:g + 1],
                    )
                rsum = rpool.tile([P, 1], FP32, tag="rsum")
                nc.vector.tensor_reduce(
                    rsum[:], rs[:], axis=mybir.AxisListType.X, op=mybir.AluOpType.add
                )
                recip = rpool.tile([P, 1], FP32, tag="recip")
                nc.vector.reciprocal(recip[:], rsum[:])

                out_ps = psA.tile([P, D], FP32, tag="psA")
                for t in range(NT):
                    pst = psT.tile([P, P], BF16, tag="pst")
                    nc.tensor.transpose(pst[:], attn[:, ts(t, P)], ident[:])
                    at = tpool.tile([P, P], BF16, tag="at")
                    nc.vector.tensor_copy(at[:], pst[:])
                    nc.tensor.matmul(
                        out_ps[:], at[:], v[t][:],
                        start=(t == 0), stop=(t == NT - 1),
                    )
                out_sb = opool.tile([P, D], FP32, tag="out")
                nc.vector.scalar_tensor_tensor(
                    out_sb[:], out_ps[:], recip[:, :1], bvb[:],
                    op0=mybir.AluOpType.mult, op1=mybir.AluOpType.add,
                )
                nc.sync.dma_start(out_d[ts(i, P), :], out_sb[:])

    nc.finalize()
    return nc


_NC_CACHE = {}


def get_nc():
    if "nc" not in _NC_CACHE:
        _NC_CACHE["nc"] = build_nc()
    return _NC_CACHE["nc"]


def prep_inputs(x1, x2, Wq, bq, Wk, bk, Wv, bv):
    bf = ml_dtypes.bfloat16
    f32 = np.float32
    x1 = np.asarray(x1, f32)
    x2 = np.asarray(x2, f32)
    shared = {
        "wqt": np.ascontiguousarray(np.asarray(Wq, f32).T).astype(bf),
        "wkt": np.ascontiguousarray(np.asarray(Wk, f32).T).astype(bf),
        "wvt": np.ascontiguousarray(np.asarray(Wv, f32).T).astype(bf),
        "bqs": np.ascontiguousarray(np.asarray(bq, f32).reshape(DC, P).T),
        "bks": np.ascontiguousarray(np.asarray(bk, f32).reshape(DC, P).T),
        "bvb": np.ascontiguousarray(
            np.broadcast_to(np.asarray(bv, f32)[None, :], (P, D))
        ),
    }
    in_maps = []
    for b in range(B):
        m = dict(shared)
        m["x1t"] = np.ascontiguousarray(x1[b].T).astype(bf)
        m["x2t"] = np.ascontiguousarray(x2[b].T).astype(bf)
        in_maps.append(m)
    return in_maps


def kernel(x1, x2, Wq, bq, Wk, bk, Wv, bv, _trace=False, _tmpdir=None):
    nc = get_nc()
    in_maps = prep_inputs(x1, x2, Wq, bq, Wk, bk, Wv, bv)
    res = run_bass_kernel_spmd(
        nc, in_maps, list(range(N_CORES)), trace=_trace, tmpdir=_tmpdir
    )
    out = np.stack([res.results[b]["out"] for b in range(B)], axis=0)
    if _trace:
        kernel.last_results = res
    return out
